# revision 33
# baseline (speedup 1.0000x reference)
"""Multi-head self-attention on 8 Trainium2 NeuronCores.

Sharding: batch (2) x head-groups (4 groups of 4 heads) -> 8 cores.
Per core: x[b] @ wq/wk/wv column slices (256 ch), 4 heads of attention,
row-parallel wo -> partial [2048, 1024] output; host sums the 4 group
partials per batch.

Design (ScalarE-exp is the binding resource: 16.7M exp elements =
128 ACTIVATE instrs of [128,1024] ~ 147us; everything else must hide
inside that):
  - Head-PAIR packing: qT/kT stored [128 part = headA(0:64)|headB(64:128),
    2048 t] bf16, NO K-padding.  Score matmuls are K=64 row-group pairs
    (tile_position (0,0)/(64,0) via base_partition) running CONCURRENTLY
    in the PE array -> 2x score throughput.
  - Slot = (pair j, th 512-block of t1, chunk i of 128 t2): packed score
    pair -> one [128,1024] EXP ACTIVATE (both heads) -> PV lagged one
    full th-WINDOW so ScalarE streams back-to-back and V production
    fits window 0's spare tensor cycles.
  - PSUM: s [128,1024]x2 (4 banks) + o A/B [128,512] (2) + proj/outproj
    [128,512]x2 (2) = 8 banks.
  - o tiles staged to SBUF right after the last PV (two quick copies) so
    the softmax normalize chain (recip -> gpsimd broadcast -> mul) runs
    off the critical path; next window's PV reuses the banks immediately.
    Norms are issued at the START of the window after their stage so the
    attnT writes always precede their outproj readers in issue order
    (issue order defines the dependency direction; reader-first is racy).
  - projections/outproj cut into ~2-matmul quanta pumped from a single
    carry-over stream into every window's spare tensor cycles; outproj
    t-ranges stream in as both pairs' attnT land, tail ranges borrow the
    freed o banks for a deeper psum pipeline.
  - single-queue input DMA in priority order (wk,wq,xT,wv,wo); prologue
    q/k groups issued kd-major so matmuls chase the arriving xT chunks.
  - everything bf16 except PSUM accum + denominators; y output bf16,
    host sums partials in f32.
  - tail: norm chains hide behind the last PV drain / outproj matmuls;
    the final outproj ranges rotate over 4 psum banks (p + freed o) with
    ystage casts split across the idle scalar engine and vector.
Measured: ~210.5-211.5us HW exec (v1 baseline: ~257us), rel err 5.8e-3.
Profile: first ACT ~30us (6.8us fixed preamble + ~14us HBM-bound input
DMA + cold-clock prologue), attention ~155us (ScalarE-bound; floor is
147us of EXP + ~6us structural proj overload in the first two windows),
tail ~25us (PV drain, last two outproj ranges, y DMA drain, teardown).
"""

import sys

sys.path.insert(0, "/opt/trn_rl_repo")

import numpy as np
import ml_dtypes
import concourse.bass as bass
import concourse.mybir as mybir
import concourse.tile as tile
from concourse import bacc
from concourse.bass_utils import run_bass_kernel_spmd

B, T, D = 2, 2048, 1024
NH = 4  # heads per core
HD = 64  # head dim
CH = NH * HD  # 256 channels per core
KD = D // 128  # 8 k-ptiles
TP = T // 128  # 16 t2 chunks
W = 512  # t1 window width
NTH = T // W  # 4 th windows
VW = HD + 1  # 65: v columns + ones column
VROW = NH * VW  # 260
VPAD = TP * VROW + 64

F32 = mybir.dt.float32
BF16 = mybir.dt.bfloat16
EXP = mybir.ActivationFunctionType.Exp

_cached_nc = None


def _wlayout(w):
    """[G*128, C] -> [128, G*C] kd-major host relayout (contiguous DMA)."""
    g = w.shape[0] // 128
    return np.ascontiguousarray(
        w.reshape(g, 128, w.shape[1]).transpose(1, 0, 2).reshape(128, -1)
    )


def _build():
    nc = bacc.Bacc(None, target_bir_lowering=False)
    xT = nc.dram_tensor("xT", [D, T], BF16, kind="ExternalInput")
    wq = nc.dram_tensor("wq", [128, KD * CH], BF16, kind="ExternalInput")
    wk = nc.dram_tensor("wk", [128, KD * CH], BF16, kind="ExternalInput")
    wv = nc.dram_tensor("wv", [128, KD * CH], BF16, kind="ExternalInput")
    wo = nc.dram_tensor("wo", [128, 2 * D], BF16, kind="ExternalInput")
    y = nc.dram_tensor("y", [T, D], BF16, kind="ExternalOutput")

    with tile.TileContext(nc) as tc:
        with (
            tc.tile_pool(name="sb", bufs=1) as sb,
            tc.tile_pool(name="pep", bufs=20) as pep,
            tc.tile_pool(name="ostg", bufs=4) as ostg,
            tc.tile_pool(name="small", bufs=2) as small,
            tc.tile_pool(name="ysp", bufs=4) as ysp,
            tc.tile_pool(name="ps_s", bufs=2, space="PSUM") as ps_s,
            tc.tile_pool(name="ps_o", bufs=1, space="PSUM") as ps_o,
            tc.tile_pool(name="ps_p", bufs=2, space="PSUM") as ps_p,
        ):
            xTt = sb.tile([128, KD * T], BF16)
            wqt = sb.tile([128, KD * CH], BF16)
            wkt = sb.tile([128, KD * CH], BF16)
            wvt = sb.tile([128, KD * CH], BF16)
            wot = sb.tile([128, 2 * D], BF16)
            qT = [sb.tile([128, T], BF16, name=f"qT{j}") for j in range(2)]
            kT = [sb.tile([128, T], BF16, name=f"kT{j}") for j in range(2)]
            vt = sb.tile([128, VPAD], BF16)
            attnT = [sb.tile([128, T], BF16, name=f"attnT{j}") for j in range(2)]

            # --- input DMAs: one sync queue, priority order.  Parallel
            # queues share HBM bandwidth, so spreading inputs only delays
            # the critical xT; wk/wq lead (prologue needs them), wv/wo
            # trail (needed later). ---
            nc.sync.dma_start(wkt[:], wk[:])
            nc.sync.dma_start(wqt[:], wq[:])
            for kd in range(KD):
                nc.sync.dma_start(
                    xTt[:, kd * T : (kd + 1) * T], xT[kd * 128 : (kd + 1) * 128, :]
                )
            nc.sync.dma_start(wvt[:], wv[:])
            nc.sync.dma_start(wot[:], wo[:])
            # ones columns of vt (offsets 64 + 65*k) + 64-col pad tail
            nc.vector.memset(
                bass.AP(vt.tensor, HD, [[VPAD, 128], [VW, NH * TP]]), 1.0
            )
            nc.vector.memset(vt[:, TP * VROW : VPAD], 1.0)

            # ---------- work-quantum generators (proj / outproj) ----------
            def qk_group(j, dst, wsb, tb):
                ps = ps_p.tile([128, W], F32, tag="p", name="pps")
                for kd in range(KD):
                    nc.tensor.matmul(
                        ps[:],
                        wsb[:, kd * CH + j * 128 : kd * CH + j * 128 + 128],
                        xTt[:, kd * T + tb * W : kd * T + (tb + 1) * W],
                        start=(kd == 0),
                        stop=(kd == KD - 1),
                    )
                    if kd % 2 == 1:
                        yield
                nc.vector.tensor_copy(dst[:, tb * W : (tb + 1) * W], ps[:])

            def v_group(tp):
                ps = ps_p.tile([128, W], F32, tag="p", name="vps")
                for kd in range(KD):
                    nc.tensor.matmul(
                        ps[:, 0:CH],
                        xTt[:, kd * T + tp * 128 : kd * T + tp * 128 + 128],
                        wvt[:, kd * CH : (kd + 1) * CH],
                        start=(kd == 0),
                        stop=(kd == KD - 1),
                    )
                    if kd % 4 == 3:
                        yield
                nc.vector.tensor_copy(
                    bass.AP(vt.tensor, tp * VROW, [[VPAD, 128], [VW, NH], [1, HD]]),
                    ps[:, 0:CH].rearrange("p (h c) -> p h c", h=NH),
                )

            def o_group(tp, ob, pool_tag=None, scalar_cast=False, dma_eng=None):
                # pool_tag lets tail groups borrow the freed o banks for a
                # deeper outproj pipeline (ps_p rotation is only 2 bufs);
                # scalar_cast/dma_eng move tail casts + final y DMAs onto
                # the post-attention idle scalar engine (hardware DGE) so
                # neither vector nor the sync DMA queue gates the drain
                if pool_tag is None:
                    ps = ps_p.tile([128, W], F32, tag="p", name="ops")
                else:
                    ps = ps_o.tile([128, W], F32, tag=pool_tag, name="ops")
                for j in range(2):
                    nc.tensor.matmul(
                        ps[:],
                        attnT[j][:, tp * 128 : tp * 128 + 128],
                        wot[:, j * D + ob * W : j * D + (ob + 1) * W],
                        start=(j == 0),
                        stop=(j == 1),
                    )
                yield
                yt = ysp.tile([128, W], BF16, tag="yt", name="yt")
                if scalar_cast:
                    nc.scalar.copy(yt[:], ps[:])
                else:
                    nc.vector.tensor_copy(yt[:], ps[:])
                (dma_eng or nc.sync).dma_start(
                    y[tp * 128 : (tp + 1) * 128, ob * W : (ob + 1) * W], yt[:]
                )

            # ---------- carry-over work stream ----------
            stream = []

            def pump(n):
                k = 0
                while k < n and stream:
                    try:
                        next(stream[0])
                        k += 1
                    except StopIteration:
                        stream.pop(0)

            def drain_stream():
                while stream:
                    try:
                        next(stream[0])
                    except StopIteration:
                        stream.pop(0)

            # ---------- attention machinery ----------
            pe_saved = {}
            o_tiles = {}
            staged = {}
            pending_norm = []

            s_tiles = {}

            def score_mm(j, th, i):
                s = ps_s.tile([128, 2 * W], F32, tag="s", name="s")
                s_tiles[(j, th, i)] = s
                for par in range(2):
                    nc.tensor.matmul(
                        s[:, par * W : (par + 1) * W],
                        kT[j][par * 64 : (par + 1) * 64, i * 128 : i * 128 + 128],
                        qT[j][par * 64 : (par + 1) * 64, th * W : (th + 1) * W],
                        start=True,
                        stop=True,
                    )

            def act_exp(j, th, i):
                s = s_tiles.pop((j, th, i))
                pe = pep.tile([128, 2 * W], BF16, tag="pe", name="pe")
                nc.scalar.activation(pe[:], s[:], EXP, scale=0.125)
                pe_saved[(j, th, i)] = pe

            def pv(j, th, i, use_p=False):
                if i == 0:
                    if use_p:
                        # last window: accumulate in the p banks (stream is
                        # empty there) so PV needs no one-window lag
                        o_tiles[(j, th)] = [
                            ps_p.tile([128, W], F32, tag="p", name="oP")
                            for _ in range(2)
                        ]
                    else:
                        o_tiles[(j, th)] = [
                            ps_o.tile([128, W], F32, tag="oA", name="oA"),
                            ps_o.tile([128, W], F32, tag="oB", name="oB"),
                        ]
                ot = o_tiles[(j, th)]
                pe = pe_saved.pop((j, th, i))
                for par in range(2):
                    hh = 2 * j + par
                    nc.tensor.matmul(
                        ot[par][:],
                        vt[:, i * VROW + hh * VW : i * VROW + hh * VW + 128],
                        pe[:, par * W : (par + 1) * W],
                        start=(i == 0),
                        stop=(i == TP - 1),
                    )

            def stage_o(j, th):
                # free the o PSUM banks fast: denom (f32) + data (bf16)
                ot = o_tiles.pop((j, th))
                st = {}
                for par in range(2):
                    den = small.tile([1, W], F32, tag="den", name="den")
                    dat = ostg.tile([64, W], BF16, tag="dat", name="dat")
                    nc.vector.tensor_copy(den[:], ot[par][64:65, :])
                    nc.vector.tensor_copy(dat[:], ot[par][0:64, :])
                    st[par] = (den, dat)
                staged[(j, th)] = st
                pending_norm.append((j, th))

            def finish_norms():
                while pending_norm:
                    j, th = pending_norm.pop(0)
                    st = staged.pop((j, th))
                    for par in range(2):
                        den, dat = st[par]
                        rt = small.tile([1, W], F32, tag="rt", name="rt")
                        Rt = small.tile([64, W], F32, tag="Rt", name="Rt")
                        # NOTE: reciprocal input must be partition-aligned
                        # with its output (partition-shifted non-copy DVE
                        # ops silently corrupt); the den copy realigns.
                        nc.vector.reciprocal_approx_fast(rt[:], den[:])
                        nc.gpsimd.partition_broadcast(Rt[:], rt[:])
                        nc.vector.tensor_mul(
                            attnT[j][par * 64 : (par + 1) * 64, th * W : (th + 1) * W],
                            dat[:],
                            Rt[:],
                        )

            def window(j, th, pv_jth, adds, per_slot, pv2=None):
                # norms first: attnT writes must be issued before any
                # freshly-added o_group readers (issue order = dep order)
                finish_norms()
                stream.extend(adds)
                # 2-slot score lookahead: S(i+2) issues right after ACT(i),
                # so every ACT finds its input scored a full slot early and
                # ScalarE streams without sem-latency stalls
                score_mm(j, th, 0)
                score_mm(j, th, 1)
                for i in range(TP):
                    act_exp(j, th, i)
                    if i + 2 < TP:
                        score_mm(j, th, i + 2)
                    if pv_jth is not None:
                        pv(pv_jth[0], pv_jth[1], i)
                    if pv2 is not None and i >= 2:
                        pv(pv2[0], pv2[1], i - 2, use_p=True)
                    pump(per_slot)
                if pv_jth is not None:
                    stage_o(*pv_jth)

            # ---------- schedule ----------
            # prologue: k0 tb0 + q0 th0 issued kd-major (matmuls chase the
            # arriving xT chunks)
            g1 = qk_group(0, kT[0], wkt, 0)
            g2 = qk_group(0, qT[0], wqt, 0)
            for _ in range(4):
                next(g1, None)
                next(g2, None)
            for g in (g1, g2):
                for _ in g:
                    pass

            window(
                0, 0, None,
                [
                    qk_group(0, kT[0], wkt, 1),
                    qk_group(0, kT[0], wkt, 2),
                    qk_group(0, kT[0], wkt, 3),
                    qk_group(0, qT[0], wqt, 1),
                ]
                + [v_group(tp) for tp in range(TP)],
                3,
            )
            window(
                0, 1, (0, 0),
                [
                    qk_group(0, qT[0], wqt, 2),
                    qk_group(0, qT[0], wqt, 3),
                    qk_group(1, kT[1], wkt, 0),
                    qk_group(1, kT[1], wkt, 1),
                ],
                1,
            )
            window(
                0, 2, (0, 1),
                [
                    qk_group(1, kT[1], wkt, 2),
                    qk_group(1, kT[1], wkt, 3),
                    qk_group(1, qT[1], wqt, 0),
                    qk_group(1, qT[1], wqt, 1),
                ],
                1,
            )
            window(0, 3, (0, 2), [qk_group(1, qT[1], wqt, 2)], 1)

            window(1, 0, (0, 3), [qk_group(1, qT[1], wqt, 3)], 1)
            window(1, 1, (1, 0), [], 1)
            window(
                1, 2, (1, 1),
                [o_group(tp, ob) for tp in range(0, 4) for ob in range(2)],
                1,
            )
            window(
                1, 3, (1, 2),
                [o_group(tp, ob) for tp in range(4, 8) for ob in range(2)],
                1,
            )
            # tail: norm (1,2) runs on vector/gpsimd WHILE the tensor
            # drains PV(1,3); then norm (1,3) hides behind the r2 outproj
            # matmuls; r3 runs over a 4-bank rotation with casts split
            # scalar/vector (scalar is idle post-attention)
            finish_norms()
            for i in range(TP):
                pv(1, 3, i)
            stage_o(1, 3)
            finish_norms()
            tail_tags = [None, None, "oA", "oB"]
            stream.extend(
                o_group(
                    tp, ob, tail_tags[(2 * tp + ob) % 4],
                    scalar_cast=(ob == 1),
                    dma_eng=(nc.scalar if tp >= 12 and ob == 0 else None),
                )
                for tp in range(8, 16)
                for ob in range(2)
            )
            drain_stream()

    nc.compile()
    return nc


def kernel(x, wq, wk, wv, wo, trace=False):
    global _cached_nc
    if _cached_nc is None:
        _cached_nc = _build()
    nc = _cached_nc

    x = np.asarray(x, dtype=np.float32)
    wq = np.asarray(wq, dtype=np.float32)
    wk = np.asarray(wk, dtype=np.float32)
    wv = np.asarray(wv, dtype=np.float32)
    wo = np.asarray(wo, dtype=np.float32)

    in_maps = []
    for c in range(8):
        b, g = c // 4, c % 4
        cs = slice(g * CH, (g + 1) * CH)
        in_maps.append(
            {
                "xT": np.ascontiguousarray(x[b].T).astype(ml_dtypes.bfloat16),
                "wq": _wlayout(wq[:, cs]).astype(ml_dtypes.bfloat16),
                "wk": _wlayout(wk[:, cs]).astype(ml_dtypes.bfloat16),
                "wv": _wlayout(wv[:, cs]).astype(ml_dtypes.bfloat16),
                "wo": _wlayout(wo[cs, :]).astype(ml_dtypes.bfloat16),
            }
        )

    # the device intermittently drops input DMAs after a prior crash,
    # yielding inf/garbage; detect the signature and retry (healthy runs
    # have |y| ~ O(1))
    for _attempt in range(4):
        res = run_bass_kernel_spmd(
            nc, in_maps, core_ids=list(range(8)), trace=trace
        )
        out = np.zeros((B, T, D), np.float32)
        for c in range(8):
            b = c // 4
            out[b] += res.results[c]["y"].astype(np.float32)
        if np.isfinite(out).all() and np.abs(out).max() < 1e3:
            break
    if trace:
        kernel.last_results = res
    return out


# revision 34
# speedup vs baseline: 1.0003x; 1.0003x over previous
"""Multi-head self-attention on 8 Trainium2 NeuronCores.

Sharding: batch (2) x head-groups (4 groups of 4 heads) -> 8 cores.
Per core: x[b] @ wq/wk/wv column slices (256 ch), 4 heads of attention,
row-parallel wo -> partial [2048, 1024] output; host sums the 4 group
partials per batch.

Design (ScalarE-exp is the binding resource: 16.7M exp elements =
128 ACTIVATE instrs of [128,1024] ~ 147us; everything else must hide
inside that):
  - Head-PAIR packing: qT/kT stored [128 part = headA(0:64)|headB(64:128),
    2048 t] bf16, NO K-padding.  Score matmuls are K=64 row-group pairs
    (tile_position (0,0)/(64,0) via base_partition) running CONCURRENTLY
    in the PE array -> 2x score throughput.
  - Slot = (pair j, th 512-block of t1, chunk i of 128 t2): packed score
    pair -> one [128,1024] EXP ACTIVATE (both heads) -> PV lagged one
    full th-WINDOW so ScalarE streams back-to-back and V production
    fits window 0's spare tensor cycles.
  - PSUM: s [128,1024]x2 (4 banks) + o A/B [128,512] (2) + proj/outproj
    [128,512]x2 (2) = 8 banks.
  - o tiles staged to SBUF right after the last PV (two quick copies) so
    the softmax normalize chain (recip -> gpsimd broadcast -> mul) runs
    off the critical path; next window's PV reuses the banks immediately.
    Norms are issued at the START of the window after their stage so the
    attnT writes always precede their outproj readers in issue order
    (issue order defines the dependency direction; reader-first is racy).
  - projections/outproj cut into ~2-matmul quanta pumped from a single
    carry-over stream into every window's spare tensor cycles; outproj
    t-ranges stream in as both pairs' attnT land, tail ranges borrow the
    freed o banks for a deeper psum pipeline.
  - single-queue input DMA in priority order (wk,wq,xT,wv,wo); prologue
    q/k groups issued kd-major so matmuls chase the arriving xT chunks.
  - everything bf16 except PSUM accum + denominators; y output bf16,
    host sums partials in f32.
  - tail: norm chains hide behind the last PV drain / outproj matmuls;
    the final outproj ranges rotate over 4 psum banks (p + freed o) with
    ystage casts split across the idle scalar engine and vector.
Measured: ~208us HW exec (v1 baseline: ~257us), rel err 5.8e-3.
Profile: first ACT ~30us (6.8us fixed preamble + ~14us HBM-bound input
DMA + cold-clock prologue), attention ~155us (ScalarE-bound; floor is
147us of EXP + ~6us structural proj overload in the first two windows),
tail ~25us (PV drain, last two outproj ranges, y DMA drain, teardown).
"""

import sys

sys.path.insert(0, "/opt/trn_rl_repo")

import numpy as np
import ml_dtypes
import concourse.bass as bass
import concourse.mybir as mybir
import concourse.tile as tile
from concourse import bacc
from concourse.bass_utils import run_bass_kernel_spmd

B, T, D = 2, 2048, 1024
NH = 4  # heads per core
HD = 64  # head dim
CH = NH * HD  # 256 channels per core
KD = D // 128  # 8 k-ptiles
TP = T // 128  # 16 t2 chunks
W = 512  # t1 window width
NTH = T // W  # 4 th windows
VW = HD + 1  # 65: v columns + ones column
VROW = NH * VW  # 260
VPAD = TP * VROW + 64

F32 = mybir.dt.float32
BF16 = mybir.dt.bfloat16
EXP = mybir.ActivationFunctionType.Exp

_cached_nc = None


def _wlayout(w):
    """[G*128, C] -> [128, G*C] kd-major host relayout (contiguous DMA)."""
    g = w.shape[0] // 128
    return np.ascontiguousarray(
        w.reshape(g, 128, w.shape[1]).transpose(1, 0, 2).reshape(128, -1)
    )


def _build():
    nc = bacc.Bacc(None, target_bir_lowering=False)
    xT = nc.dram_tensor("xT", [D, T], BF16, kind="ExternalInput")
    wq = nc.dram_tensor("wq", [128, KD * CH], BF16, kind="ExternalInput")
    wk = nc.dram_tensor("wk", [128, KD * CH], BF16, kind="ExternalInput")
    wv = nc.dram_tensor("wv", [128, KD * CH], BF16, kind="ExternalInput")
    wo = nc.dram_tensor("wo", [128, 2 * D], BF16, kind="ExternalInput")
    y = nc.dram_tensor("y", [T, D], BF16, kind="ExternalOutput")

    with tile.TileContext(nc) as tc:
        with (
            tc.tile_pool(name="sb", bufs=1) as sb,
            tc.tile_pool(name="pep", bufs=20) as pep,
            tc.tile_pool(name="ostg", bufs=4) as ostg,
            tc.tile_pool(name="small", bufs=2) as small,
            tc.tile_pool(name="ysp", bufs=4) as ysp,
            tc.tile_pool(name="ps_s", bufs=2, space="PSUM") as ps_s,
            tc.tile_pool(name="ps_o", bufs=1, space="PSUM") as ps_o,
            tc.tile_pool(name="ps_p", bufs=2, space="PSUM") as ps_p,
        ):
            xTt = sb.tile([128, KD * T], BF16)
            wqt = sb.tile([128, KD * CH], BF16)
            wkt = sb.tile([128, KD * CH], BF16)
            wvt = sb.tile([128, KD * CH], BF16)
            wot = sb.tile([128, 2 * D], BF16)
            qT = [sb.tile([128, T], BF16, name=f"qT{j}") for j in range(2)]
            kT = [sb.tile([128, T], BF16, name=f"kT{j}") for j in range(2)]
            vt = sb.tile([128, VPAD], BF16)
            attnT = [sb.tile([128, T], BF16, name=f"attnT{j}") for j in range(2)]

            # --- input DMAs: one sync queue, priority order.  Parallel
            # queues share HBM bandwidth, so spreading inputs only delays
            # the critical xT; wk/wq lead (prologue needs them), wv/wo
            # trail (needed later). ---
            nc.sync.dma_start(wkt[:], wk[:])
            nc.sync.dma_start(wqt[:], wq[:])
            for kd in range(KD):
                nc.sync.dma_start(
                    xTt[:, kd * T : (kd + 1) * T], xT[kd * 128 : (kd + 1) * 128, :]
                )
            nc.sync.dma_start(wvt[:], wv[:])
            nc.sync.dma_start(wot[:], wo[:])
            # ones columns of vt (offsets 64 + 65*k) + 64-col pad tail
            nc.vector.memset(
                bass.AP(vt.tensor, HD, [[VPAD, 128], [VW, NH * TP]]), 1.0
            )
            nc.vector.memset(vt[:, TP * VROW : VPAD], 1.0)

            # ---------- work-quantum generators (proj / outproj) ----------
            def qk_group(j, dst, wsb, tb):
                ps = ps_p.tile([128, W], F32, tag="p", name="pps")
                for kd in range(KD):
                    nc.tensor.matmul(
                        ps[:],
                        wsb[:, kd * CH + j * 128 : kd * CH + j * 128 + 128],
                        xTt[:, kd * T + tb * W : kd * T + (tb + 1) * W],
                        start=(kd == 0),
                        stop=(kd == KD - 1),
                    )
                    if kd % 2 == 1:
                        yield
                nc.vector.tensor_copy(dst[:, tb * W : (tb + 1) * W], ps[:])

            def v_group(tp):
                ps = ps_p.tile([128, W], F32, tag="p", name="vps")
                for kd in range(KD):
                    nc.tensor.matmul(
                        ps[:, 0:CH],
                        xTt[:, kd * T + tp * 128 : kd * T + tp * 128 + 128],
                        wvt[:, kd * CH : (kd + 1) * CH],
                        start=(kd == 0),
                        stop=(kd == KD - 1),
                    )
                    if kd % 4 == 3:
                        yield
                nc.vector.tensor_copy(
                    bass.AP(vt.tensor, tp * VROW, [[VPAD, 128], [VW, NH], [1, HD]]),
                    ps[:, 0:CH].rearrange("p (h c) -> p h c", h=NH),
                )

            def o_group(tp, ob, pool_tag=None, scalar_cast=False, dma_eng=None):
                # pool_tag lets tail groups borrow the freed o banks for a
                # deeper outproj pipeline (ps_p rotation is only 2 bufs);
                # scalar_cast/dma_eng move tail casts + final y DMAs onto
                # the post-attention idle scalar engine (hardware DGE) so
                # neither vector nor the sync DMA queue gates the drain
                if pool_tag is None:
                    ps = ps_p.tile([128, W], F32, tag="p", name="ops")
                else:
                    ps = ps_o.tile([128, W], F32, tag=pool_tag, name="ops")
                for j in range(2):
                    nc.tensor.matmul(
                        ps[:],
                        attnT[j][:, tp * 128 : tp * 128 + 128],
                        wot[:, j * D + ob * W : j * D + (ob + 1) * W],
                        start=(j == 0),
                        stop=(j == 1),
                    )
                yield
                yt = ysp.tile([128, W], BF16, tag="yt", name="yt")
                if scalar_cast:
                    nc.scalar.copy(yt[:], ps[:])
                else:
                    nc.vector.tensor_copy(yt[:], ps[:])
                (dma_eng or nc.sync).dma_start(
                    y[tp * 128 : (tp + 1) * 128, ob * W : (ob + 1) * W], yt[:]
                )

            # ---------- carry-over work stream ----------
            stream = []

            def pump(n):
                k = 0
                while k < n and stream:
                    try:
                        next(stream[0])
                        k += 1
                    except StopIteration:
                        stream.pop(0)

            def drain_stream():
                while stream:
                    try:
                        next(stream[0])
                    except StopIteration:
                        stream.pop(0)

            # ---------- attention machinery ----------
            pe_saved = {}
            o_tiles = {}
            staged = {}
            pending_norm = []

            s_tiles = {}

            def score_mm(j, th, i):
                s = ps_s.tile([128, 2 * W], F32, tag="s", name="s")
                s_tiles[(j, th, i)] = s
                for par in range(2):
                    nc.tensor.matmul(
                        s[:, par * W : (par + 1) * W],
                        kT[j][par * 64 : (par + 1) * 64, i * 128 : i * 128 + 128],
                        qT[j][par * 64 : (par + 1) * 64, th * W : (th + 1) * W],
                        start=True,
                        stop=True,
                    )

            def act_exp(j, th, i):
                s = s_tiles.pop((j, th, i))
                pe = pep.tile([128, 2 * W], BF16, tag="pe", name="pe")
                nc.scalar.activation(pe[:], s[:], EXP, scale=0.125)
                pe_saved[(j, th, i)] = pe

            def pv(j, th, i, use_p=False):
                if i == 0:
                    if use_p:
                        # last window: accumulate in the p banks (stream is
                        # empty there) so PV needs no one-window lag
                        o_tiles[(j, th)] = [
                            ps_p.tile([128, W], F32, tag="p", name="oP")
                            for _ in range(2)
                        ]
                    else:
                        o_tiles[(j, th)] = [
                            ps_o.tile([128, W], F32, tag="oA", name="oA"),
                            ps_o.tile([128, W], F32, tag="oB", name="oB"),
                        ]
                ot = o_tiles[(j, th)]
                pe = pe_saved.pop((j, th, i))
                for par in range(2):
                    hh = 2 * j + par
                    nc.tensor.matmul(
                        ot[par][:],
                        vt[:, i * VROW + hh * VW : i * VROW + hh * VW + 128],
                        pe[:, par * W : (par + 1) * W],
                        start=(i == 0),
                        stop=(i == TP - 1),
                    )

            def stage_o(j, th):
                # free the o PSUM banks fast: denom (f32) + data (bf16)
                ot = o_tiles.pop((j, th))
                st = {}
                for par in range(2):
                    den = small.tile([1, W], F32, tag="den", name="den")
                    dat = ostg.tile([64, W], BF16, tag="dat", name="dat")
                    nc.vector.tensor_copy(den[:], ot[par][64:65, :])
                    nc.vector.tensor_copy(dat[:], ot[par][0:64, :])
                    st[par] = (den, dat)
                staged[(j, th)] = st
                pending_norm.append((j, th))

            def finish_norms():
                while pending_norm:
                    j, th = pending_norm.pop(0)
                    st = staged.pop((j, th))
                    for par in range(2):
                        den, dat = st[par]
                        rt = small.tile([1, W], F32, tag="rt", name="rt")
                        Rt = small.tile([64, W], F32, tag="Rt", name="Rt")
                        # NOTE: reciprocal input must be partition-aligned
                        # with its output (partition-shifted non-copy DVE
                        # ops silently corrupt); the den copy realigns.
                        nc.vector.reciprocal_approx_fast(rt[:], den[:])
                        nc.gpsimd.partition_broadcast(Rt[:], rt[:])
                        nc.vector.tensor_mul(
                            attnT[j][par * 64 : (par + 1) * 64, th * W : (th + 1) * W],
                            dat[:],
                            Rt[:],
                        )

            def window(j, th, pv_jth, adds, per_slot, pv2=None):
                # norms first: attnT writes must be issued before any
                # freshly-added o_group readers (issue order = dep order)
                finish_norms()
                stream.extend(adds)
                # 2-slot score lookahead: S(i+2) issues right after ACT(i),
                # so every ACT finds its input scored a full slot early and
                # ScalarE streams without sem-latency stalls
                score_mm(j, th, 0)
                score_mm(j, th, 1)
                for i in range(TP):
                    act_exp(j, th, i)
                    if i + 2 < TP:
                        score_mm(j, th, i + 2)
                    if pv_jth is not None:
                        pv(pv_jth[0], pv_jth[1], i)
                    if pv2 is not None and i >= 2:
                        pv(pv2[0], pv2[1], i - 2, use_p=True)
                    pump(per_slot)
                if pv_jth is not None:
                    stage_o(*pv_jth)

            # ---------- schedule ----------
            # prologue: k0 tb0 + q0 th0 issued kd-major (matmuls chase the
            # arriving xT chunks)
            g1 = qk_group(0, kT[0], wkt, 0)
            g2 = qk_group(0, qT[0], wqt, 0)
            for _ in range(4):
                next(g1, None)
                next(g2, None)
            for g in (g1, g2):
                for _ in g:
                    pass

            window(
                0, 0, None,
                [
                    qk_group(0, kT[0], wkt, 1),
                    qk_group(0, kT[0], wkt, 2),
                    qk_group(0, kT[0], wkt, 3),
                    qk_group(0, qT[0], wqt, 1),
                ]
                + [v_group(tp) for tp in range(TP)],
                3,
            )
            window(
                0, 1, (0, 0),
                [
                    qk_group(0, qT[0], wqt, 2),
                    qk_group(0, qT[0], wqt, 3),
                    qk_group(1, kT[1], wkt, 0),
                    qk_group(1, kT[1], wkt, 1),
                ],
                1,
            )
            window(
                0, 2, (0, 1),
                [
                    qk_group(1, kT[1], wkt, 2),
                    qk_group(1, kT[1], wkt, 3),
                    qk_group(1, qT[1], wqt, 0),
                    qk_group(1, qT[1], wqt, 1),
                ],
                1,
            )
            window(0, 3, (0, 2), [qk_group(1, qT[1], wqt, 2)], 1)

            window(1, 0, (0, 3), [qk_group(1, qT[1], wqt, 3)], 1)
            window(1, 1, (1, 0), [], 1)
            window(
                1, 2, (1, 1),
                [o_group(tp, ob) for tp in range(0, 4) for ob in range(2)],
                1,
            )
            window(
                1, 3, (1, 2),
                [o_group(tp, ob) for tp in range(4, 8) for ob in range(2)],
                1,
            )
            # tail: norm (1,2) runs on vector/gpsimd WHILE the tensor
            # drains PV(1,3); then norm (1,3) hides behind the r2 outproj
            # matmuls; r3 runs over a 4-bank rotation with casts split
            # scalar/vector (scalar is idle post-attention)
            finish_norms()
            for i in range(TP):
                pv(1, 3, i)
            stage_o(1, 3)
            finish_norms()
            tail_tags = [None, None, "oA", "oB"]
            stream.extend(
                o_group(tp, ob, tail_tags[(2 * tp + ob) % 4], scalar_cast=(ob == 1))
                for tp in range(8, 16)
                for ob in range(2)
            )
            drain_stream()

    nc.compile()
    return nc


def kernel(x, wq, wk, wv, wo, trace=False):
    global _cached_nc
    if _cached_nc is None:
        _cached_nc = _build()
    nc = _cached_nc

    x = np.asarray(x, dtype=np.float32)
    wq = np.asarray(wq, dtype=np.float32)
    wk = np.asarray(wk, dtype=np.float32)
    wv = np.asarray(wv, dtype=np.float32)
    wo = np.asarray(wo, dtype=np.float32)

    in_maps = []
    for c in range(8):
        b, g = c // 4, c % 4
        cs = slice(g * CH, (g + 1) * CH)
        in_maps.append(
            {
                "xT": np.ascontiguousarray(x[b].T).astype(ml_dtypes.bfloat16),
                "wq": _wlayout(wq[:, cs]).astype(ml_dtypes.bfloat16),
                "wk": _wlayout(wk[:, cs]).astype(ml_dtypes.bfloat16),
                "wv": _wlayout(wv[:, cs]).astype(ml_dtypes.bfloat16),
                "wo": _wlayout(wo[cs, :]).astype(ml_dtypes.bfloat16),
            }
        )

    # the device intermittently drops input DMAs after a prior crash,
    # yielding inf/garbage; detect the signature and retry (healthy runs
    # have |y| ~ O(1))
    for _attempt in range(4):
        res = run_bass_kernel_spmd(
            nc, in_maps, core_ids=list(range(8)), trace=trace
        )
        out = np.zeros((B, T, D), np.float32)
        for c in range(8):
            b = c // 4
            out[b] += res.results[c]["y"].astype(np.float32)
        if np.isfinite(out).all() and np.abs(out).max() < 1e3:
            break
    if trace:
        kernel.last_results = res
    return out


# revision 35
# speedup vs baseline: 1.0191x; 1.0188x over previous
"""Multi-head self-attention on 8 Trainium2 NeuronCores.

Sharding: batch (2) x head-groups (4 groups of 4 heads) -> 8 cores.
Per core: x[b] @ wq/wk/wv column slices (256 ch), 4 heads of attention,
row-parallel wo -> partial [2048, 1024] output; host sums the 4 group
partials per batch.

Design (ScalarE-exp is the binding resource: 16.7M exp elements =
128 ACTIVATE instrs of [128,1024] ~ 147us; everything else must hide
inside that):
  - Head-PAIR packing: qT/kT stored [128 part = headA(0:64)|headB(64:128),
    2048 t] bf16, NO K-padding.  Score matmuls are K=64 row-group pairs
    (tile_position (0,0)/(64,0) via base_partition) running CONCURRENTLY
    in the PE array -> 2x score throughput.
  - Slot = (pair j, th 512-block of t1, chunk i of 128 t2): packed score
    pair -> one [128,1024] EXP ACTIVATE (both heads) -> PV lagged one
    full th-WINDOW so ScalarE streams back-to-back and V production
    fits window 0's spare tensor cycles.
  - PSUM: s [128,1024]x2 (4 banks) + o A/B [128,512] (2) + proj/outproj
    [128,512]x2 (2) = 8 banks.
  - o tiles staged to SBUF right after the last PV (two quick copies) so
    the softmax normalize chain (recip -> gpsimd broadcast -> mul) runs
    off the critical path; next window's PV reuses the banks immediately.
    Norms are issued at the START of the window after their stage so the
    attnT writes always precede their outproj readers in issue order
    (issue order defines the dependency direction; reader-first is racy).
  - projections/outproj cut into ~2-matmul quanta pumped from a single
    carry-over stream into every window's spare tensor cycles; outproj
    t-ranges stream in as both pairs' attnT land, tail ranges borrow the
    freed o banks for a deeper psum pipeline.
  - single-queue input DMA in priority order (wk,wq,xT,wv,wo); prologue
    q/k groups issued kd-major so matmuls chase the arriving xT chunks.
  - everything bf16 except PSUM accum + denominators; y output bf16,
    host sums partials in f32.
  - tail: norm chains hide behind the last PV drain / outproj matmuls;
    the final outproj ranges rotate over 4 psum banks (p + freed o) with
    ystage casts split across the idle scalar engine and vector.
Measured: ~208us HW exec (v1 baseline: ~257us), rel err 5.8e-3.
Profile: first ACT ~30us (6.8us fixed preamble + ~14us HBM-bound input
DMA + cold-clock prologue), attention ~155us (ScalarE-bound; floor is
147us of EXP + ~6us structural proj overload in the first two windows),
tail ~25us (PV drain, last two outproj ranges, y DMA drain, teardown).
"""

import sys

sys.path.insert(0, "/opt/trn_rl_repo")

import numpy as np
import ml_dtypes
import concourse.bass as bass
import concourse.mybir as mybir
import concourse.tile as tile
from concourse import bacc
from concourse.bass_utils import run_bass_kernel_spmd

B, T, D = 2, 2048, 1024
NH = 4  # heads per core
HD = 64  # head dim
CH = NH * HD  # 256 channels per core
KD = D // 128  # 8 k-ptiles
TP = T // 128  # 16 t2 chunks
W = 512  # t1 window width
NTH = T // W  # 4 th windows
VW = HD + 1  # 65: v columns + ones column
VROW = NH * VW  # 260
VPAD = TP * VROW + 64

F32 = mybir.dt.float32
BF16 = mybir.dt.bfloat16
EXP = mybir.ActivationFunctionType.Exp

_cached_nc = None


def _wlayout(w):
    """[G*128, C] -> [128, G*C] kd-major host relayout (contiguous DMA)."""
    g = w.shape[0] // 128
    return np.ascontiguousarray(
        w.reshape(g, 128, w.shape[1]).transpose(1, 0, 2).reshape(128, -1)
    )


def _build():
    nc = bacc.Bacc(None, target_bir_lowering=False)
    xT = nc.dram_tensor("xT", [D, T], BF16, kind="ExternalInput")
    wq = nc.dram_tensor("wq", [128, KD * CH], BF16, kind="ExternalInput")
    wk = nc.dram_tensor("wk", [128, KD * CH], BF16, kind="ExternalInput")
    wv = nc.dram_tensor("wv", [128, KD * CH], BF16, kind="ExternalInput")
    wo = nc.dram_tensor("wo", [128, 2 * D], BF16, kind="ExternalInput")
    y = nc.dram_tensor("y", [T, D], BF16, kind="ExternalOutput")

    with tile.TileContext(nc) as tc:
        with (
            tc.tile_pool(name="sb", bufs=1) as sb,
            tc.tile_pool(name="pep", bufs=20) as pep,
            tc.tile_pool(name="ostg", bufs=4) as ostg,
            tc.tile_pool(name="small", bufs=2) as small,
            tc.tile_pool(name="ysp", bufs=4) as ysp,
            tc.tile_pool(name="ps_s", bufs=2, space="PSUM") as ps_s,
            tc.tile_pool(name="ps_o", bufs=1, space="PSUM") as ps_o,
            tc.tile_pool(name="ps_p", bufs=2, space="PSUM") as ps_p,
        ):
            xTt = sb.tile([128, KD * T], BF16)
            wqt = sb.tile([128, KD * CH], BF16)
            wkt = sb.tile([128, KD * CH], BF16)
            wvt = sb.tile([128, KD * CH], BF16)
            wot = sb.tile([128, 2 * D], BF16)
            qT = [sb.tile([128, T], BF16, name=f"qT{j}") for j in range(2)]
            kT = [sb.tile([128, T], BF16, name=f"kT{j}") for j in range(2)]
            vt = sb.tile([128, VPAD], BF16)
            attnT = [sb.tile([128, T], BF16, name=f"attnT{j}") for j in range(2)]

            # --- input DMAs: one sync queue, priority order.  Parallel
            # queues share HBM bandwidth, so spreading inputs only delays
            # the critical xT; wk/wq lead (prologue needs them), wv/wo
            # trail (needed later). ---
            nc.sync.dma_start(wkt[:], wk[:])
            nc.sync.dma_start(wqt[:], wq[:])
            for kd in range(KD):
                nc.sync.dma_start(
                    xTt[:, kd * T : (kd + 1) * T], xT[kd * 128 : (kd + 1) * 128, :]
                )
            nc.sync.dma_start(wvt[:], wv[:])
            nc.sync.dma_start(wot[:], wo[:])
            # ones columns of vt (offsets 64 + 65*k) + 64-col pad tail
            nc.vector.memset(
                bass.AP(vt.tensor, HD, [[VPAD, 128], [VW, NH * TP]]), 1.0
            )
            nc.vector.memset(vt[:, TP * VROW : VPAD], 1.0)

            # ---------- work-quantum generators (proj / outproj) ----------
            def qk_group(j, dst, wsb, tb):
                ps = ps_p.tile([128, W], F32, tag="p", name="pps")
                for kd in range(KD):
                    nc.tensor.matmul(
                        ps[:],
                        wsb[:, kd * CH + j * 128 : kd * CH + j * 128 + 128],
                        xTt[:, kd * T + tb * W : kd * T + (tb + 1) * W],
                        start=(kd == 0),
                        stop=(kd == KD - 1),
                    )
                    if kd % 2 == 1:
                        yield
                nc.vector.tensor_copy(dst[:, tb * W : (tb + 1) * W], ps[:])

            def v_group(tp):
                ps = ps_p.tile([128, W], F32, tag="p", name="vps")
                for kd in range(KD):
                    nc.tensor.matmul(
                        ps[:, 0:CH],
                        xTt[:, kd * T + tp * 128 : kd * T + tp * 128 + 128],
                        wvt[:, kd * CH : (kd + 1) * CH],
                        start=(kd == 0),
                        stop=(kd == KD - 1),
                    )
                    if kd % 4 == 3:
                        yield
                nc.vector.tensor_copy(
                    bass.AP(vt.tensor, tp * VROW, [[VPAD, 128], [VW, NH], [1, HD]]),
                    ps[:, 0:CH].rearrange("p (h c) -> p h c", h=NH),
                )

            def o_pair(tp, tags=(None, None), scalar_cast=False):
                # one full output row [128, 1024] per generator: two psum
                # accumulations, two casts, ONE y DMA (descriptor setup is
                # ~640ns regardless of size, so merging halves the sync
                # queue's descriptor load that paces the tail drain).
                # tags lets tail pairs borrow the freed o banks; scalar_cast
                # puts one cast on the post-attention idle scalar engine.
                yt = ysp.tile([128, 2 * W], BF16, tag="yt2", name="yt")
                for ob in range(2):
                    if tags[ob] is None:
                        ps = ps_p.tile([128, W], F32, tag="p", name="ops")
                    else:
                        ps = ps_o.tile([128, W], F32, tag=tags[ob], name="ops")
                    for j in range(2):
                        nc.tensor.matmul(
                            ps[:],
                            attnT[j][:, tp * 128 : tp * 128 + 128],
                            wot[:, j * D + ob * W : j * D + (ob + 1) * W],
                            start=(j == 0),
                            stop=(j == 1),
                        )
                    yield
                    if scalar_cast and ob == 1:
                        nc.scalar.copy(yt[:, ob * W : (ob + 1) * W], ps[:])
                    else:
                        nc.vector.tensor_copy(yt[:, ob * W : (ob + 1) * W], ps[:])
                nc.sync.dma_start(y[tp * 128 : (tp + 1) * 128, :], yt[:])

            # ---------- carry-over work stream ----------
            stream = []

            def pump(n):
                k = 0
                while k < n and stream:
                    try:
                        next(stream[0])
                        k += 1
                    except StopIteration:
                        stream.pop(0)

            def drain_stream():
                while stream:
                    try:
                        next(stream[0])
                    except StopIteration:
                        stream.pop(0)

            # ---------- attention machinery ----------
            pe_saved = {}
            o_tiles = {}
            staged = {}
            pending_norm = []

            s_tiles = {}

            def score_mm(j, th, i):
                s = ps_s.tile([128, 2 * W], F32, tag="s", name="s")
                s_tiles[(j, th, i)] = s
                for par in range(2):
                    nc.tensor.matmul(
                        s[:, par * W : (par + 1) * W],
                        kT[j][par * 64 : (par + 1) * 64, i * 128 : i * 128 + 128],
                        qT[j][par * 64 : (par + 1) * 64, th * W : (th + 1) * W],
                        start=True,
                        stop=True,
                    )

            def act_exp(j, th, i):
                s = s_tiles.pop((j, th, i))
                pe = pep.tile([128, 2 * W], BF16, tag="pe", name="pe")
                nc.scalar.activation(pe[:], s[:], EXP, scale=0.125)
                pe_saved[(j, th, i)] = pe

            def pv(j, th, i, use_p=False):
                if i == 0:
                    if use_p:
                        # last window: accumulate in the p banks (stream is
                        # empty there) so PV needs no one-window lag
                        o_tiles[(j, th)] = [
                            ps_p.tile([128, W], F32, tag="p", name="oP")
                            for _ in range(2)
                        ]
                    else:
                        o_tiles[(j, th)] = [
                            ps_o.tile([128, W], F32, tag="oA", name="oA"),
                            ps_o.tile([128, W], F32, tag="oB", name="oB"),
                        ]
                ot = o_tiles[(j, th)]
                pe = pe_saved.pop((j, th, i))
                for par in range(2):
                    hh = 2 * j + par
                    nc.tensor.matmul(
                        ot[par][:],
                        vt[:, i * VROW + hh * VW : i * VROW + hh * VW + 128],
                        pe[:, par * W : (par + 1) * W],
                        start=(i == 0),
                        stop=(i == TP - 1),
                    )

            def stage_o(j, th):
                # free the o PSUM banks fast: denom (f32) + data (bf16)
                ot = o_tiles.pop((j, th))
                st = {}
                for par in range(2):
                    den = small.tile([1, W], F32, tag="den", name="den")
                    dat = ostg.tile([64, W], BF16, tag="dat", name="dat")
                    nc.vector.tensor_copy(den[:], ot[par][64:65, :])
                    nc.vector.tensor_copy(dat[:], ot[par][0:64, :])
                    st[par] = (den, dat)
                staged[(j, th)] = st
                pending_norm.append((j, th))

            def finish_norms():
                while pending_norm:
                    j, th = pending_norm.pop(0)
                    st = staged.pop((j, th))
                    for par in range(2):
                        den, dat = st[par]
                        rt = small.tile([1, W], F32, tag="rt", name="rt")
                        Rt = small.tile([64, W], F32, tag="Rt", name="Rt")
                        # NOTE: reciprocal input must be partition-aligned
                        # with its output (partition-shifted non-copy DVE
                        # ops silently corrupt); the den copy realigns.
                        nc.vector.reciprocal_approx_fast(rt[:], den[:])
                        nc.gpsimd.partition_broadcast(Rt[:], rt[:])
                        nc.vector.tensor_mul(
                            attnT[j][par * 64 : (par + 1) * 64, th * W : (th + 1) * W],
                            dat[:],
                            Rt[:],
                        )

            def window(j, th, pv_jth, adds, per_slot, pv2=None):
                # norms first: attnT writes must be issued before any
                # freshly-added o_group readers (issue order = dep order)
                finish_norms()
                stream.extend(adds)
                # 2-slot score lookahead: S(i+2) issues right after ACT(i),
                # so every ACT finds its input scored a full slot early and
                # ScalarE streams without sem-latency stalls
                score_mm(j, th, 0)
                score_mm(j, th, 1)
                for i in range(TP):
                    act_exp(j, th, i)
                    if i + 2 < TP:
                        score_mm(j, th, i + 2)
                    if pv_jth is not None:
                        pv(pv_jth[0], pv_jth[1], i)
                    if pv2 is not None and i >= 2:
                        pv(pv2[0], pv2[1], i - 2, use_p=True)
                    pump(per_slot)
                if pv_jth is not None:
                    stage_o(*pv_jth)

            # ---------- schedule ----------
            # prologue: k0 tb0 + q0 th0 issued kd-major (matmuls chase the
            # arriving xT chunks)
            g1 = qk_group(0, kT[0], wkt, 0)
            g2 = qk_group(0, qT[0], wqt, 0)
            for _ in range(4):
                next(g1, None)
                next(g2, None)
            for g in (g1, g2):
                for _ in g:
                    pass

            window(
                0, 0, None,
                [
                    qk_group(0, kT[0], wkt, 1),
                    qk_group(0, kT[0], wkt, 2),
                    qk_group(0, kT[0], wkt, 3),
                    qk_group(0, qT[0], wqt, 1),
                ]
                + [v_group(tp) for tp in range(TP)],
                3,
            )
            window(
                0, 1, (0, 0),
                [
                    qk_group(0, qT[0], wqt, 2),
                    qk_group(0, qT[0], wqt, 3),
                    qk_group(1, kT[1], wkt, 0),
                    qk_group(1, kT[1], wkt, 1),
                ],
                1,
            )
            window(
                0, 2, (0, 1),
                [
                    qk_group(1, kT[1], wkt, 2),
                    qk_group(1, kT[1], wkt, 3),
                    qk_group(1, qT[1], wqt, 0),
                    qk_group(1, qT[1], wqt, 1),
                ],
                1,
            )
            window(0, 3, (0, 2), [qk_group(1, qT[1], wqt, 2)], 1)

            window(1, 0, (0, 3), [qk_group(1, qT[1], wqt, 3)], 1)
            window(1, 1, (1, 0), [], 1)
            window(
                1, 2, (1, 1),
                [o_pair(tp) for tp in range(0, 4)],
                1,
            )
            window(
                1, 3, (1, 2),
                [o_pair(tp) for tp in range(4, 8)],
                1,
            )
            # tail: norm (1,2) runs on vector/gpsimd WHILE the tensor
            # drains PV(1,3); then norm (1,3) hides behind the r2 outproj
            # matmuls; r3 runs over a 4-bank rotation with casts split
            # scalar/vector (scalar is idle post-attention)
            finish_norms()
            for i in range(TP):
                pv(1, 3, i)
            stage_o(1, 3)
            finish_norms()
            stream.extend(
                o_pair(
                    tp,
                    tags=((None, None) if tp % 2 == 0 else ("oA", "oB")),
                    scalar_cast=True,
                )
                for tp in range(8, 16)
            )
            drain_stream()

    nc.compile()
    return nc


def kernel(x, wq, wk, wv, wo, trace=False):
    global _cached_nc
    if _cached_nc is None:
        _cached_nc = _build()
    nc = _cached_nc

    x = np.asarray(x, dtype=np.float32)
    wq = np.asarray(wq, dtype=np.float32)
    wk = np.asarray(wk, dtype=np.float32)
    wv = np.asarray(wv, dtype=np.float32)
    wo = np.asarray(wo, dtype=np.float32)

    in_maps = []
    for c in range(8):
        b, g = c // 4, c % 4
        cs = slice(g * CH, (g + 1) * CH)
        in_maps.append(
            {
                "xT": np.ascontiguousarray(x[b].T).astype(ml_dtypes.bfloat16),
                "wq": _wlayout(wq[:, cs]).astype(ml_dtypes.bfloat16),
                "wk": _wlayout(wk[:, cs]).astype(ml_dtypes.bfloat16),
                "wv": _wlayout(wv[:, cs]).astype(ml_dtypes.bfloat16),
                "wo": _wlayout(wo[cs, :]).astype(ml_dtypes.bfloat16),
            }
        )

    # the device intermittently drops input DMAs after a prior crash,
    # yielding inf/garbage; detect the signature and retry (healthy runs
    # have |y| ~ O(1))
    for _attempt in range(4):
        res = run_bass_kernel_spmd(
            nc, in_maps, core_ids=list(range(8)), trace=trace
        )
        out = np.zeros((B, T, D), np.float32)
        for c in range(8):
            b = c // 4
            out[b] += res.results[c]["y"].astype(np.float32)
        if np.isfinite(out).all() and np.abs(out).max() < 1e3:
            break
    if trace:
        kernel.last_results = res
    return out
